# revision 49
# baseline (speedup 1.0000x reference)
"""Trainium2 Bass kernel for a decoder layer (DecoderAttention).

Math (reference):
    x   = tok_emb[target_tokens] + pos_emb[:S]                   # [B,S,H]
    x   = attn(x, x,   Wq_s, Wk_s, Wv_s, causal=True)            # self-attn
    x   = attn(x, enc, Wq_c, Wk_c, Wv_c, causal=False)           # cross-attn
    out = x @ Wout + bout                                        # [B,S,V]
with B=4, S=512, ENC=1024, H=1024, V=32000, single-head over full hidden dim.

Sharding: 8 cores = 4 batches x 2 vocab halves for the output GEMM (phase C).
The two cores sharing a batch each compute the attention phases (A/B) for
HALF the queries (rank r owns q in [r*256, r*256+256)), then exchange their
att2e halves with a pairwise AllGather ([[0,1],[2,3],[4,5],[6,7]]) at the
B->C boundary.  This halves the per-core attention work (which was previously
duplicated across the vocab halves) at the cost of a ~1MB pair exchange.
The program is SPMD-uniform: each core's role lives entirely in its input
DATA (its query slice xq, and a causal mask tensor), never in the
instruction stream.

Algebraic restructure (exact, cuts PE work ~25% vs the naive chain).  With
P1/P2 the normalized attention matrices, the network is
    out = P2 @ enc @ Wv_c @ Wout + bout',  with
    P2  = softmax(att1 @ Wq_c @ Wk_c^T @ enc^T),
    att1 = P1 @ x @ Wv_s,   P1 = softmax_causal(x @ Wq_s @ Wk_s^T @ x^T)
so the device only ever computes, per batch:
    qkT   = x @ Wqk_s                    (Wqk_s = Wq_s Wk_s^T, host GEMM)
    P1_un = exp(scale * x qkT^T)         (causal-masked, unnormalized)
    att1e = (P1_un @ x) * rr1            (rr1 = row reciprocal sums)
    qk2T  = att1e @ W1                   (W1 = Wv_s Wq_c Wk_c^T, host GEMM)
    P2_un = exp(scale * enc qk2T^T)
    att2e = (P2_un @ enc) * rr2
    out   = att2e @ W2 + bout'           (W2 = Wv_c Wout, host GEMM)
Biases fold exactly: k-side biases are softmax no-ops; q-side biases become
per-key logit offsets (sbias/cbias, host-precomputed, applied as the exp's
per-partition bias); v-side biases ride Wqk_c/Wout into cbias/bout'.

All matmul operands are bf16, accumulating in f32 PSUM.  Softmax runs on
TRANSPOSED scores s^T[k, q], so no PE transposes are needed; exp needs no
max subtraction (scores*scale ~ N(0,~2)).  Row sums over k come from a
ones-column matmul into a [1, SQ] PSUM tile; GpSimd broadcasts the
reciprocal; normalization is folded into the att1e/att2e copies.  Causality
is one multiplicative 0/1 mask tensor (host-built per core role) applied to
all four exp outputs.

Startup is chip-HBM-bound (8 cores pull inputs at once), so only the
critical ~1MB (xq + wqks half0) loads at t0; everything else is deferred
via tile_wait_until to stream under compute.  The first GEMM runs hi-outer
in two ho-half passes so its first matmul needs just one xq + one wqks
chunk.  Output is stored bf16 and upcast on the host.
"""

import numpy as np
import ml_dtypes

import concourse.mybir as mybir
import concourse.tile as tile
from concourse import bacc, bass
from concourse.tile import add_dep_helper

P = 128
B, S, ENC, H, V = 4, 512, 1024, 1024, 32000
HT = H // P            # 8 h-tiles of 128
SC = S // P            # 4 key chunks of 128
EC = ENC // P          # 8 encoder chunks
SQ = S // 2            # 256 queries owned per core in phases A/B
VSH = V // 2           # 16000 vocab columns per core
NV = 500               # vocab tile: 32*500 = 16000
NVC = VSH // NV        # 32
N_PRE = 12             # W2 chunks resident for the local-first sweep
NCORES = 8
F32 = mybir.dt.float32
BF16 = mybir.dt.bfloat16
MMDT = BF16
SCALE = 1.0 / np.sqrt(H)
BF16NP = ml_dtypes.bfloat16
PAIRS = [[0, 1], [2, 3], [4, 5], [6, 7]]


def build_program(has_sb=False, has_cb=False, has_bout=False):
    """Trace + compile the single-core SPMD program. Returns nc."""
    nc = bacc.Bacc("TRN2", target_bir_lowering=False, debug=False,
                   num_devices=NCORES)

    # host-retiled inputs (see _host_prep for layouts)
    xq_d = nc.dram_tensor("xQ", [P, HT, SQ], MMDT, kind="ExternalInput")
    xt_d = nc.dram_tensor("xR", [P, HT, S], MMDT, kind="ExternalInput")
    xs_d = nc.dram_tensor("xS", [P, HT, SC, P], MMDT, kind="ExternalInput")
    encT_d = nc.dram_tensor("encTR", [P, HT, ENC], MMDT, kind="ExternalInput")
    encS_d = nc.dram_tensor("encSR", [P, HT, EC, P], MMDT, kind="ExternalInput")
    mask_d = nc.dram_tensor("maskA", [P, SC, SQ], MMDT, kind="ExternalInput")
    peer_d = nc.dram_tensor("peerv", [1, 2], mybir.dt.int32, kind="ExternalInput")
    cc_in_d = nc.dram_tensor("ccin", [P, HT, SQ], BF16)
    cc_out_d = nc.dram_tensor("ccout", [2, P, HT, SQ], BF16)
    wqks_d = nc.dram_tensor("WqkS", [P, 2, HT, 512], MMDT, kind="ExternalInput")
    w1_d = nc.dram_tensor("W1", [P, 2, HT, 512], MMDT, kind="ExternalInput")
    w2_d = nc.dram_tensor("W2R", [NVC, P, HT, NV], MMDT, kind="ExternalInput")
    # bf16 output (host upcasts): halves store traffic + end-of-kernel drain
    out_d = nc.dram_tensor("out", [S, VSH], BF16, kind="ExternalOutput")
    if has_sb:
        sbias_d = nc.dram_tensor("sbias", [P, SC], F32, kind="ExternalInput")
    if has_cb:
        cbias_d = nc.dram_tensor("cbias", [P, EC], F32, kind="ExternalInput")
    if has_bout:
        bout_d = nc.dram_tensor("bout", [VSH], MMDT, kind="ExternalInput")

    Exp = mybir.ActivationFunctionType.Exp
    MUL = mybir.AluOpType.mult

    rwait_mm = []
    with tile.TileContext(nc) as tc:
        with tc.tile_pool(name="persist", bufs=1) as persist, \
             tc.tile_pool(name="stat", bufs=2) as stat, \
             tc.tile_pool(name="psum", bufs=4, space="PSUM") as psum, \
             tc.tile_pool(name="psum_s", bufs=2, space="PSUM") as psum_s, \
             tc.tile_pool(name="psum_r", bufs=2, space="PSUM") as psum_r:

            ones_col = persist.tile([P, 1], MMDT, name="ones_col")
            nc.vector.memset(ones_col[:, :], 1.0)

            att2eT = [persist.tile([P, S], MMDT, name=f"a2e{i}")
                      for i in range(HT)]

            # ---- W2 prefetch pool; batches issued behind each softmax
            # broadcast on the gpsimd queue ----
            wprep = tc.alloc_tile_pool(name="wpre", bufs=1)
            wpre = []

            def prefetch_w2(n):
                for _ in range(n):
                    i = len(wpre)
                    t = wprep.tile([P, HT, NV], MMDT, name=f"wpre{i}")
                    nc.gpsimd.dma_start(out=t[:, :, :], in_=w2_d[i, :, :, :])
                    wpre.append(t)

            # weight staging (2 rotating whole-weight tiles); released after
            # phase B so phase C's output staging fits (LIFO above wpre)
            wbig = tc.alloc_tile_pool(name="wbig", bufs=2)

            # att1e: [h, q-half]; pool released after phase B
            att1p = tc.alloc_tile_pool(name="att1p", bufs=1)
            att1eT = [att1p.tile([P, SQ], MMDT, name=f"a1e{i}")
                      for i in range(HT)]

            def wcol(w, hi, ho):
                # lhsT [128, 128] slice for h_out chunk ho
                return w[:, ho // 4, hi, (ho % 4) * P:(ho % 4 + 1) * P]

            def proj_T(dst_tiles, w_t, rhs_of_hi, rr_t=None):
                """dst[ho][128, SQ] = (W.T @ rhs)[ho-chunk] (* rr broadcast)."""
                for ho in range(HT):
                    ps = psum.tile([P, 512], F32, tag="acc")
                    for hi in range(HT):
                        nc.tensor.matmul(
                            out=ps[:, :SQ],
                            lhsT=wcol(w_t, hi, ho),
                            rhs=rhs_of_hi(hi),
                            start=(hi == 0), stop=(hi == HT - 1),
                        )
                    if rr_t is not None:
                        nc.vector.tensor_tensor(out=dst_tiles[ho][:, :],
                                                in0=ps[:, :SQ], in1=rr_t[:, :],
                                                op=MUL)
                    else:
                        nc.vector.tensor_copy(out=dst_tiles[ho][:, :],
                                              in_=ps[:, :SQ])

            def make_RR(p_tiles, RR_t, npre):
                """RR_t[128, SQ] = 1 / colsums of unnormalized transposed p."""
                n = len(p_tiles)
                rs = psum_r.tile([1, SQ], F32, tag="rs")
                for c in range(n):
                    nc.tensor.matmul(
                        out=rs[0:1, :], lhsT=ones_col[:, :],
                        rhs=p_tiles[c][:, :],
                        start=(c == 0), stop=(c == n - 1),
                    )
                rr = stat.tile([1, SQ], F32, tag="rr")
                nc.vector.reciprocal(out=rr[0:1, :], in_=rs[0:1, :])
                nc.gpsimd.partition_broadcast(RR_t[:, :], rr[0:1, :], channels=P)
                prefetch_w2(min(npre, N_PRE - len(wpre)))

            # ---------------- Phase A: self-attention ----------------
            with tc.tile_pool(name="phA", bufs=1) as pA:

                # critical t0 loads: xq (proj rhs) + wqks half 0 chunks
                xq = pA.tile([P, HT, SQ], MMDT, name="xq")
                nc.sync.dma_start(out=xq[:, :, :], in_=xq_d[:, :, :])
                wqk = wbig.tile([P, 2, HT, 512], MMDT, tag="w", name="wqks")
                for c in range(2):
                    nc.scalar.dma_start(out=wqk[:, 0, 4 * c:4 * c + 4, :],
                                        in_=wqks_d[:, 0, 4 * c:4 * c + 4, :])
                # wqks half 1 (proj pass 2) on the fast gpsimd ring
                nc.gpsimd.dma_start(out=wqk[:, 1, :, :], in_=wqks_d[:, 1, :, :])
                # xt (score keys) right behind xq on sync
                xt = pA.tile([P, HT, S], MMDT, name="xt")
                for c in range(2):
                    nc.sync.dma_start(out=xt[:, 4 * c:4 * c + 4, :],
                                      in_=xt_d[:, 4 * c:4 * c + 4, :])
                msk = pA.tile([P, SC, SQ], MMDT, name="msk")
                nc.scalar.dma_start(out=msk[:, :, :], in_=mask_d[:, :, :])
                peer_sb = persist.tile([1, 2], mybir.dt.int32, name="peer_sb")
                nc.sync.dma_start(out=peer_sb[:, :], in_=peer_d[:, :])
                rpeer = nc.sync.alloc_register("rpeer")
                nc.sync.reg_load(rpeer, peer_sb[0:1, 0:1])
                rpeer_sv = nc.sync.snap(rpeer, min_val=0, max_val=1)
                sb = None
                if has_sb:
                    sb = pA.tile([P, SC], F32, name="sb")
                    nc.scalar.dma_start(out=sb[:, :], in_=sbias_d[:, :])

                # deferred loads, staggered by first-use time
                xs = pA.tile([P, HT, SC, P], MMDT, name="xs")
                with tc.tile_wait_until(0.005):
                    nc.gpsimd.dma_start(out=xs[:, :, :, :],
                                        in_=xs_d[:, :, :, :])
                    w1 = wbig.tile([P, 2, HT, 512], MMDT, tag="w", name="w1")
                    nc.gpsimd.dma_start(out=w1[:, 0, :, :], in_=w1_d[:, 0, :, :])
                    nc.gpsimd.dma_start(out=w1[:, 1, :, :], in_=w1_d[:, 1, :, :])

                qkT = [pA.tile([P, SQ], MMDT, name=f"qkT{i}") for i in range(HT)]
                pT = [pA.tile([P, SQ], MMDT, name=f"pT{i}") for i in range(SC)]
                # RR outlives phase A (consumed by qk2T's copies in phase B)
                RR = persist.tile([P, SQ], F32, name="RR")

                # hi-outer proj in two ho-half passes (4 PSUM accumulators
                # each): the first matmul depends on one xq + one wqks chunk
                for half in range(2):
                    ps4 = [psum.tile([P, 512], F32, tag="acc",
                                     name=f"ps4_{half}_{j}") for j in range(4)]
                    for hi in range(HT):
                        for j in range(4):
                            nc.tensor.matmul(
                                out=ps4[j][:, :SQ],
                                lhsT=wcol(wqk, hi, half * 4 + j),
                                rhs=xq[:, hi, :],
                                start=(hi == 0), stop=(hi == HT - 1),
                            )
                    for j in range(4):
                        nc.vector.tensor_copy(out=qkT[half * 4 + j][:, :],
                                              in_=ps4[j][:, :SQ])

                # transposed scores per 128-key chunk; exp; causal mask
                for kc in range(SC):
                    sp = psum_s.tile([P, 512], F32, tag="sT")
                    for hi in range(HT):
                        nc.tensor.matmul(
                            out=sp[:, :SQ],
                            lhsT=xt[:, hi, kc * P:(kc + 1) * P],
                            rhs=qkT[hi][:, :],
                            start=(hi == 0), stop=(hi == HT - 1),
                        )
                    nc.scalar.activation(
                        pT[kc][:, :], sp[:, :SQ], Exp, scale=SCALE,
                        bias=sb[:, kc:kc + 1] if sb is not None else 0.0)
                    nc.vector.tensor_tensor(
                        out=pT[kc][:, :], in0=pT[kc][:, :],
                        in1=msk[:, kc, :], op=MUL)

                # att1e[q, h] = (P1_un @ x)[q, h]; transposed accum.  The
                # 1/rowsum normalization is deferred to qk2T's copies.
                for ho in range(HT):
                    ps = psum.tile([P, 512], F32, tag="acc")
                    for kc in range(SC):
                        nc.tensor.matmul(
                            out=ps[:, :SQ], lhsT=xs[:, ho, kc, :],
                            rhs=pT[kc][:, :], start=(kc == 0),
                            stop=(kc == SC - 1))
                    nc.vector.tensor_copy(out=att1eT[ho][:, :], in_=ps[:, :SQ])
                make_RR(pT, RR, 4)

            # ---------------- Phase B: cross-attention ----------------
            with tc.tile_pool(name="phB", bufs=1) as pB:

                cb = None
                if has_cb:
                    cb = pB.tile([P, EC], F32, name="cb")
                    nc.sync.dma_start(out=cb[:, :], in_=cbias_d[:, :])

                qk2T = [pB.tile([P, SQ], MMDT, name=f"qk2T{i}")
                        for i in range(HT)]
                p2T = [pB.tile([P, SQ], MMDT, name=f"p2T{i}") for i in range(EC)]
                RR2 = pB.tile([P, SQ], F32, name="RR2")

                # encS [e-part, hc, ec, j] for att2e; encT [h-part, e] for
                # scores.  encT's scope closes first (LIFO) to free SBUF.
                with tc.tile_pool(name="phBeS", bufs=1) as pBs:
                    encS = pBs.tile([P, HT, EC, P], MMDT, name="encS")
                    with tc.tile_wait_until(0.011):
                        nc.sync.dma_start(out=encS[:, :, :, :],
                                          in_=encS_d[:, :, :, :])

                    with tc.tile_pool(name="phBeT", bufs=1) as pBt:
                        encT = pBt.tile([P, HT, ENC], MMDT, name="encT")
                        with tc.tile_wait_until(0.008):
                            nc.scalar.dma_start(out=encT[:, :, :],
                                                in_=encT_d[:, :, :])

                        proj_T(qk2T, w1, lambda hi: att1eT[hi][:, :], rr_t=RR)

                        # transposed cross scores per 128-key (encoder) chunk
                        for ec in range(EC):
                            sp = psum_s.tile([P, 512], F32, tag="sT")
                            for hi in range(HT):
                                nc.tensor.matmul(
                                    out=sp[:, :SQ],
                                    lhsT=encT[:, hi, ec * P:(ec + 1) * P],
                                    rhs=qk2T[hi][:, :],
                                    start=(hi == 0), stop=(hi == HT - 1),
                                )
                            nc.scalar.activation(
                                p2T[ec][:, :], sp[:, :SQ], Exp, scale=SCALE,
                                bias=cb[:, ec:ec + 1] if cb is not None else 0.0)
                        make_RR(p2T, RR2, 4)

                    # att2e[q, h] = (P2_un @ enc) * rr2 for OUR query half,
                    # written straight into att2eT cols [0, SQ) and stored
                    # to DRAM for a pairwise AllGather; phase C's local-first
                    # sweep hides the collective's latency.
                    st_insts = []
                    for ho in range(HT):
                        ps = psum.tile([P, 512], F32, tag="acc")
                        for ec in range(EC):
                            nc.tensor.matmul(
                                out=ps[:, :SQ],
                                lhsT=encS[:, ho, ec, :],
                                rhs=p2T[ec][:, :],
                                start=(ec == 0), stop=(ec == EC - 1),
                            )
                        nc.vector.tensor_tensor(
                            out=att2eT[ho][:, 0:SQ], in0=ps[:, :SQ],
                            in1=RR2[:, :], op=MUL)
                        eng = nc.sync if ho % 2 == 0 else nc.scalar
                        st = eng.dma_start(out=cc_in_d[:, ho, :],
                                           in_=att2eT[ho][:, 0:SQ])
                        st_insts.append(st)
                    cc = nc.gpsimd.collective_compute(
                        "AllGather", mybir.AluOpType.bypass,
                        replica_groups=PAIRS,
                        ins=[cc_in_d.ap()], outs=[cc_out_d.ap()],
                    )
                    for st in st_insts:
                        add_dep_helper(cc.ins, st.ins, True,
                                       "cc reads att2e half stores")
                    # peer section of the gather -> att2eT cols [SQ, 2*SQ);
                    # the section index is the peer's rank, from input data
                    for ho in range(HT):
                        ld = nc.sync.dma_start(
                            out=att2eT[ho][:, SQ:2 * SQ],
                            in_=cc_out_d[rpeer_sv, :, ho, :])
                        add_dep_helper(ld.ins, cc.ins, True,
                                       "att2e load reads gather output")

            att1p.release()
            wbig.release()
            prefetch_w2(N_PRE - len(wpre))

            # ---------------- Phase C: output projection ----------------
            # out = att2e @ W2 (+ bout').  Local-first sweep: the first
            # N_PRE vocab chunks run query chunks 0/1 (locally computed
            # columns) first, hiding the pairwise AllGather latency; their
            # W2 tiles stay resident so the qc2/3 return pass is free.
            GRP = 2
            order = [(vc, qc) for vc in range(N_PRE) for qc in (0, 1)]
            order += [(vc, qc) for vc in range(N_PRE) for qc in (2, 3)]
            order += [(vc, qc) for vc in range(N_PRE, NVC) for qc in range(SC)]
            with tc.tile_pool(name="phC_w", bufs=4) as pW, \
                 tc.tile_pool(name="phC_o", bufs=4) as pO:

                ones_t = None
                if has_bout:
                    ones_t = persist.tile([1, P], MMDT, name="ones")
                    nc.vector.memset(ones_t[:, :], 1.0)

                osb = [None] * SC
                wt = None
                bo = None
                for vc, qc in order:
                    g = vc % GRP
                    if vc < N_PRE:
                        wt = wpre[vc]
                    elif qc == 0:
                        wt = pW.tile([P, HT, NV], MMDT, tag="wt")
                        eng = nc.scalar if vc % 2 == 0 else nc.gpsimd
                        eng.dma_start(out=wt[:, :, :], in_=w2_d[vc, :, :, :])
                    if has_bout and qc == 0:
                        bo = pW.tile([1, NV], MMDT, tag="bo")
                        nc.gpsimd.dma_start(out=bo[:, :],
                                            in_=bout_d[vc * NV:(vc + 1) * NV][None, :])
                    if g == 0:
                        osb[qc] = pO.tile([P, GRP * NV], BF16, tag=f"osb{qc}",
                                          name=f"osb{qc}_{vc}")
                    ps = psum.tile([P, NV], F32, tag="acc")
                    for hi in range(HT):
                        last = (hi == HT - 1) and not has_bout
                        nc.tensor.matmul(
                            out=ps[:, :],
                            lhsT=att2eT[hi][:, qc * P:(qc + 1) * P],
                            rhs=wt[:, hi, :],
                            start=(hi == 0), stop=last,
                        )
                    if has_bout:
                        nc.tensor.matmul(
                            out=ps[:, :], lhsT=ones_t[:, :], rhs=bo[:, :],
                            start=False, stop=True,
                        )
                    nc.vector.tensor_copy(
                        out=osb[qc][:, g * NV:(g + 1) * NV], in_=ps[:, :])
                    if g == GRP - 1:
                        v0 = (vc - g) * NV
                        # final groups alternate stores onto sync (idle by
                        # then) so the last stores drain in parallel
                        eng = (nc.sync if (vc >= 24 and qc % 2 == 1)
                               else nc.scalar)
                        eng.dma_start(
                            out=out_d[qc * P:(qc + 1) * P, v0:v0 + GRP * NV],
                            in_=osb[qc][:, :],
                        )
            wprep.release()
    # phase C's first remote-column matmul waits for all 8 peer att2e
    # sends (each +16 on rdma_rsem); added after TileContext scheduling

    nc.compile()
    return nc


def _retile_w(w):
    """[H, H] -> [128, 2, 8, 512] matching wcol's SBUF layout, contiguous."""
    return np.ascontiguousarray(
        w.reshape(HT, P, 2, 512).transpose(1, 2, 0, 3)).astype(BF16NP)


def _host_prep(inputs):
    """Numpy-side sharding/layout prep. Returns (in_maps, flags)."""
    enc = np.asarray(inputs["encoder_outputs"], dtype=np.float32)
    tok = np.asarray(inputs["target_tokens"]).astype(np.int64)
    tok_emb = np.asarray(inputs["tok_emb"], dtype=np.float32)
    pos_emb = np.asarray(inputs["pos_emb"], dtype=np.float32)
    x0 = tok_emb[tok] + pos_emb[:S][None, :, :]          # [B,S,H]

    W = {k: np.asarray(inputs[k], dtype=np.float32)
         for k in ("Wq_s", "Wk_s", "Wv_s", "Wq_c", "Wk_c", "Wv_c", "Wout")}
    bs = {k: np.asarray(inputs[k], dtype=np.float32)
          for k in ("bq_s", "bk_s", "bv_s", "bq_c", "bk_c", "bv_c", "bout")}

    wqk_c = W["Wq_c"] @ W["Wk_c"].T
    wqks = _retile_w(W["Wq_s"] @ W["Wk_s"].T)
    w1 = _retile_w(W["Wv_s"] @ wqk_c)
    w2 = W["Wv_c"] @ W["Wout"]                           # [H, V] host GEMM

    # exact bias folds: k-side biases are softmax no-ops; v-side biases ride
    # the fused weights into cbias / bout'
    bout_eff = bs["bout"] + bs["bv_c"] @ W["Wout"]
    has_sb = bool(np.any(bs["bq_s"]))
    has_cb = bool(np.any(bs["bq_c"]) or np.any(bs["bv_s"]))
    has_bout = bool(np.any(bout_eff))

    # causal masks in TRANSPOSED coords, per core rank r (q in [r*SQ, ..)):
    # mask[p, kc, j] = 1 iff global query (r*SQ + j) >= global key (kc*128+p)
    jj = np.arange(SQ)[None, None, :]
    pp = np.arange(P)[:, None, None]
    kk = np.arange(SC)[None, :, None]
    masks = [((r * SQ + jj) >= (kk * P + pp)).astype(BF16NP) for r in range(2)]

    in_maps = []
    for c in range(NCORES):
        b, r = c // 2, c % 2
        xb, eb = x0[b], enc[b]
        xh = xb[r * SQ:(r + 1) * SQ]                     # its query half
        # W2 half retiled to [vc, p, hi, j] == the SBUF tile layout
        wh = w2[:, r * VSH:(r + 1) * VSH].reshape(HT, P, NVC, NV)
        w2R = np.ascontiguousarray(wh.transpose(2, 1, 0, 3)).astype(BF16NP)
        m = {
            "xQ": np.ascontiguousarray(
                xh.reshape(SQ, HT, P).transpose(2, 1, 0)).astype(BF16NP),
            "xR": np.ascontiguousarray(
                xb.reshape(S, HT, P).transpose(2, 1, 0)).astype(BF16NP),
            "xS": np.ascontiguousarray(
                xb.reshape(SC, P, HT, P).transpose(1, 2, 0, 3)).astype(BF16NP),
            "encTR": np.ascontiguousarray(
                eb.reshape(ENC, HT, P).transpose(2, 1, 0)).astype(BF16NP),
            "encSR": np.ascontiguousarray(
                eb.reshape(EC, P, HT, P).transpose(1, 2, 0, 3)).astype(BF16NP),
            "maskA": masks[r],
            "peerv": np.array([[(c ^ 1) % 2, 0]], dtype=np.int32),
            "WqkS": wqks, "W1": w1, "W2R": w2R,
        }
        if has_sb:
            sbias = SCALE * ((bs["bq_s"] @ W["Wk_s"].T) @ xb.T)     # [S]
            m["sbias"] = np.ascontiguousarray(
                sbias.reshape(SC, P).T.astype(np.float32))
        if has_cb:
            cbias = SCALE * (((bs["bq_c"] @ W["Wk_c"].T)
                              + bs["bv_s"] @ wqk_c) @ eb.T)         # [ENC]
            m["cbias"] = np.ascontiguousarray(
                cbias.reshape(EC, P).T.astype(np.float32))
        if has_bout:
            m["bout"] = np.ascontiguousarray(
                bout_eff[r * VSH:(r + 1) * VSH]).astype(BF16NP)
        in_maps.append(m)
    return in_maps, (has_sb, has_cb, has_bout)


def assemble_output(results):
    out = np.empty((B, S, V), dtype=np.float32)
    for c in range(NCORES):
        b, vh = c // 2, c % 2
        r = results[c]["out"].astype(np.float32)
        if vh == 1:
            # att2eT cols (hence out rows) are in slot order: own query
            # half first.  Rank 1 owns q [256, 512) -> swap the halves.
            r = np.concatenate([r[SQ:], r[:SQ]], axis=0)
        out[b, :, vh * VSH:(vh + 1) * VSH] = r
    return out


def kernel(**inputs):
    from concourse.bass_utils import run_bass_kernel_spmd
    in_maps, (has_sb, has_cb, has_bout) = _host_prep(inputs)
    nc = build_program(has_sb=has_sb, has_cb=has_cb, has_bout=has_bout)
    res = run_bass_kernel_spmd(nc, in_maps, list(range(NCORES)))
    return assemble_output(res.results)


# revision 50
# speedup vs baseline: 1.1141x; 1.1141x over previous
"""Trainium2 Bass kernel for a decoder layer (DecoderAttention).

Math (reference):
    x   = tok_emb[target_tokens] + pos_emb[:S]                   # [B,S,H]
    x   = attn(x, x,   Wq_s, Wk_s, Wv_s, causal=True)            # self-attn
    x   = attn(x, enc, Wq_c, Wk_c, Wv_c, causal=False)           # cross-attn
    out = x @ Wout + bout                                        # [B,S,V]
with B=4, S=512, ENC=1024, H=1024, V=32000, single-head over full hidden dim.

Sharding: 8 cores = 4 batches x 2 vocab halves for the output GEMM (phase C).
The two cores sharing a batch each compute the attention phases (A/B) for
HALF the queries (rank r owns q in [r*256, r*256+256)), then exchange their
att2e halves with a pairwise AllGather ([[0,1],[2,3],[4,5],[6,7]]) at the
B->C boundary.  This halves the per-core attention work (which was previously
duplicated across the vocab halves) at the cost of a ~1MB pair exchange.
The program is SPMD-uniform: each core's role lives entirely in its input
DATA (its query slice xq, and a causal mask tensor), never in the
instruction stream.

Algebraic restructure (exact, cuts PE work ~25% vs the naive chain).  With
P1/P2 the normalized attention matrices, the network is
    out = P2 @ enc @ Wv_c @ Wout + bout',  with
    P2  = softmax(att1 @ Wq_c @ Wk_c^T @ enc^T),
    att1 = P1 @ x @ Wv_s,   P1 = softmax_causal(x @ Wq_s @ Wk_s^T @ x^T)
so the device only ever computes, per batch:
    qkT   = x @ Wqk_s                    (Wqk_s = Wq_s Wk_s^T, host GEMM)
    P1_un = exp(scale * x qkT^T)         (causal-masked, unnormalized)
    att1e = (P1_un @ x) * rr1            (rr1 = row reciprocal sums)
    qk2T  = att1e @ W1                   (W1 = Wv_s Wq_c Wk_c^T, host GEMM)
    P2_un = exp(scale * enc qk2T^T)
    att2e = (P2_un @ enc) * rr2
    out   = att2e @ W2 + bout'           (W2 = Wv_c Wout, host GEMM)
Biases fold exactly: k-side biases are softmax no-ops; q-side biases become
per-key logit offsets (sbias/cbias, host-precomputed, applied as the exp's
per-partition bias); v-side biases ride Wqk_c/Wout into cbias/bout'.

All matmul operands are bf16, accumulating in f32 PSUM.  Softmax runs on
TRANSPOSED scores s^T[k, q], so no PE transposes are needed; exp needs no
max subtraction (scores*scale ~ N(0,~2)).  Row sums over k come from a
ones-column matmul into a [1, SQ] PSUM tile; GpSimd broadcasts the
reciprocal; normalization is folded into the att1e/att2e copies.  Causality
is one multiplicative 0/1 mask tensor (host-built per core role) applied to
all four exp outputs.

Startup is chip-HBM-bound (8 cores pull inputs at once), so only the
critical ~1MB (xq + wqks half0) loads at t0; everything else is deferred
via tile_wait_until to stream under compute.  The first GEMM runs hi-outer
in two ho-half passes so its first matmul needs just one xq + one wqks
chunk.  Output is stored bf16 and upcast on the host.
"""

import numpy as np
import ml_dtypes

import concourse.mybir as mybir
import concourse.tile as tile
from concourse import bacc, bass
from concourse.tile import add_dep_helper

P = 128
B, S, ENC, H, V = 4, 512, 1024, 1024, 32000
HT = H // P            # 8 h-tiles of 128
SC = S // P            # 4 key chunks of 128
EC = ENC // P          # 8 encoder chunks
SQ = S // 2            # 256 queries owned per core in phases A/B
VSH = V // 2           # 16000 vocab columns per core
NV = 500               # vocab tile: 32*500 = 16000
NVC = VSH // NV        # 32
N_PRE = 12             # W2 chunks resident for the local-first sweep
NCORES = 8
F32 = mybir.dt.float32
BF16 = mybir.dt.bfloat16
MMDT = BF16
SCALE = 1.0 / np.sqrt(H)
BF16NP = ml_dtypes.bfloat16
PAIRS = [[0, 1], [2, 3], [4, 5], [6, 7]]


def build_program(has_sb=False, has_cb=False, has_bout=False):
    """Trace + compile the single-core SPMD program. Returns nc."""
    nc = bacc.Bacc("TRN2", target_bir_lowering=False, debug=False,
                   num_devices=NCORES)

    # host-retiled inputs (see _host_prep for layouts)
    xq_d = nc.dram_tensor("xQ", [P, HT, SQ], MMDT, kind="ExternalInput")
    xt_d = nc.dram_tensor("xR", [P, HT, S], MMDT, kind="ExternalInput")
    xs_d = nc.dram_tensor("xS", [P, HT, SC, P], MMDT, kind="ExternalInput")
    encT_d = nc.dram_tensor("encTR", [P, HT, ENC], MMDT, kind="ExternalInput")
    encS_d = nc.dram_tensor("encSR", [P, HT, EC, P], MMDT, kind="ExternalInput")
    mask_d = nc.dram_tensor("maskA", [P, SC, SQ], MMDT, kind="ExternalInput")
    peer_d = nc.dram_tensor("peerv", [1, 2], mybir.dt.int32, kind="ExternalInput")
    cc_in_d = nc.dram_tensor("ccin", [P, HT, SQ], BF16)
    cc_out_d = nc.dram_tensor("ccout", [2, P, HT, SQ], BF16)
    wqks_d = nc.dram_tensor("WqkS", [P, 2, HT, 512], MMDT, kind="ExternalInput")
    w1_d = nc.dram_tensor("W1", [P, 2, HT, 512], MMDT, kind="ExternalInput")
    w2_d = nc.dram_tensor("W2R", [NVC, P, HT, NV], MMDT, kind="ExternalInput")
    # bf16 output (host upcasts): halves store traffic + end-of-kernel drain
    out_d = nc.dram_tensor("out", [S, VSH], BF16, kind="ExternalOutput")
    if has_sb:
        sbias_d = nc.dram_tensor("sbias", [P, SC], F32, kind="ExternalInput")
    if has_cb:
        cbias_d = nc.dram_tensor("cbias", [P, EC], F32, kind="ExternalInput")
    if has_bout:
        bout_d = nc.dram_tensor("bout", [VSH], MMDT, kind="ExternalInput")

    Exp = mybir.ActivationFunctionType.Exp
    MUL = mybir.AluOpType.mult

    rwait_mm = []
    with tile.TileContext(nc) as tc:
        with tc.tile_pool(name="persist", bufs=1) as persist, \
             tc.tile_pool(name="stat", bufs=2) as stat, \
             tc.tile_pool(name="psum", bufs=4, space="PSUM") as psum, \
             tc.tile_pool(name="psum_s", bufs=2, space="PSUM") as psum_s, \
             tc.tile_pool(name="psum_r", bufs=2, space="PSUM") as psum_r:

            ones_col = persist.tile([P, 1], MMDT, name="ones_col")
            nc.vector.memset(ones_col[:, :], 1.0)

            att2eT = [persist.tile([P, S], MMDT, name=f"a2e{i}")
                      for i in range(HT)]

            # ---- W2 prefetch pool; batches issued behind each softmax
            # broadcast on the gpsimd queue ----
            wprep = tc.alloc_tile_pool(name="wpre", bufs=1)
            wpre = []

            def prefetch_w2(n):
                for _ in range(n):
                    i = len(wpre)
                    t = wprep.tile([P, HT, NV], MMDT, name=f"wpre{i}")
                    nc.gpsimd.dma_start(out=t[:, :, :], in_=w2_d[i, :, :, :])
                    wpre.append(t)

            # weight staging (2 rotating whole-weight tiles); released after
            # phase B so phase C's output staging fits (LIFO above wpre)
            wbig = tc.alloc_tile_pool(name="wbig", bufs=2)

            # att1e: [h, q-half]; pool released after phase B
            att1p = tc.alloc_tile_pool(name="att1p", bufs=1)
            att1eT = [att1p.tile([P, SQ], MMDT, name=f"a1e{i}")
                      for i in range(HT)]

            def wcol(w, hi, ho):
                # lhsT [128, 128] slice for h_out chunk ho
                return w[:, ho // 4, hi, (ho % 4) * P:(ho % 4 + 1) * P]

            def proj_T(dst_tiles, w_t, rhs_of_hi, rr_t=None):
                """dst[ho][128, SQ] = (W.T @ rhs)[ho-chunk] (* rr broadcast)."""
                for ho in range(HT):
                    ps = psum.tile([P, 512], F32, tag="acc")
                    for hi in range(HT):
                        nc.tensor.matmul(
                            out=ps[:, :SQ],
                            lhsT=wcol(w_t, hi, ho),
                            rhs=rhs_of_hi(hi),
                            start=(hi == 0), stop=(hi == HT - 1),
                        )
                    if rr_t is not None:
                        nc.vector.tensor_tensor(out=dst_tiles[ho][:, :],
                                                in0=ps[:, :SQ], in1=rr_t[:, :],
                                                op=MUL)
                    else:
                        nc.vector.tensor_copy(out=dst_tiles[ho][:, :],
                                              in_=ps[:, :SQ])

            def make_RR(p_tiles, RR_t, npre):
                """RR_t[128, SQ] = 1 / colsums of unnormalized transposed p."""
                n = len(p_tiles)
                rs = psum_r.tile([1, SQ], F32, tag="rs")
                for c in range(n):
                    nc.tensor.matmul(
                        out=rs[0:1, :], lhsT=ones_col[:, :],
                        rhs=p_tiles[c][:, :],
                        start=(c == 0), stop=(c == n - 1),
                    )
                rr = stat.tile([1, SQ], F32, tag="rr")
                nc.vector.reciprocal(out=rr[0:1, :], in_=rs[0:1, :])
                nc.gpsimd.partition_broadcast(RR_t[:, :], rr[0:1, :], channels=P)
                prefetch_w2(min(npre, N_PRE - len(wpre)))

            # ---------------- Phase A: self-attention ----------------
            with tc.tile_pool(name="phA", bufs=1) as pA:

                # critical t0 loads: xq (proj rhs) + wqks half 0 chunks
                xq = pA.tile([P, HT, SQ], MMDT, name="xq")
                nc.sync.dma_start(out=xq[:, :, :], in_=xq_d[:, :, :])
                wqk = wbig.tile([P, 2, HT, 512], MMDT, tag="w", name="wqks")
                nc.scalar.dma_start(out=wqk[:, 0, 0:4, :],
                                    in_=wqks_d[:, 0, 0:4, :])
                nc.gpsimd.dma_start(out=wqk[:, 0, 4:8, :],
                                    in_=wqks_d[:, 0, 4:8, :])
                # wqks half 1 (proj pass 2) behind half 0 on gpsimd
                nc.gpsimd.dma_start(out=wqk[:, 1, :, :], in_=wqks_d[:, 1, :, :])
                # xt (score keys) right behind xq on sync
                xt = pA.tile([P, HT, S], MMDT, name="xt")
                for c in range(2):
                    nc.sync.dma_start(out=xt[:, 4 * c:4 * c + 4, :],
                                      in_=xt_d[:, 4 * c:4 * c + 4, :])
                msk = pA.tile([P, SC, SQ], MMDT, name="msk")
                nc.scalar.dma_start(out=msk[:, :, :], in_=mask_d[:, :, :])
                peer_sb = persist.tile([1, 2], mybir.dt.int32, name="peer_sb")
                nc.sync.dma_start(out=peer_sb[:, :], in_=peer_d[:, :])
                rpeer = nc.sync.alloc_register("rpeer")
                nc.sync.reg_load(rpeer, peer_sb[0:1, 0:1])
                rpeer_sv = nc.sync.snap(rpeer, min_val=0, max_val=1)
                sb = None
                if has_sb:
                    sb = pA.tile([P, SC], F32, name="sb")
                    nc.scalar.dma_start(out=sb[:, :], in_=sbias_d[:, :])

                # deferred loads, staggered by first-use time
                xs = pA.tile([P, HT, SC, P], MMDT, name="xs")
                with tc.tile_wait_until(0.005):
                    nc.gpsimd.dma_start(out=xs[:, :, :, :],
                                        in_=xs_d[:, :, :, :])
                    w1 = wbig.tile([P, 2, HT, 512], MMDT, tag="w", name="w1")
                    nc.gpsimd.dma_start(out=w1[:, 0, :, :], in_=w1_d[:, 0, :, :])
                    nc.gpsimd.dma_start(out=w1[:, 1, :, :], in_=w1_d[:, 1, :, :])

                qkT = [pA.tile([P, SQ], MMDT, name=f"qkT{i}") for i in range(HT)]
                pT = [pA.tile([P, SQ], MMDT, name=f"pT{i}") for i in range(SC)]
                # RR outlives phase A (consumed by qk2T's copies in phase B)
                RR = persist.tile([P, SQ], F32, name="RR")

                # hi-outer proj in two ho-half passes (4 PSUM accumulators
                # each): the first matmul depends on one xq + one wqks chunk
                for half in range(2):
                    ps4 = [psum.tile([P, 512], F32, tag="acc",
                                     name=f"ps4_{half}_{j}") for j in range(4)]
                    for hi in range(HT):
                        for j in range(4):
                            nc.tensor.matmul(
                                out=ps4[j][:, :SQ],
                                lhsT=wcol(wqk, hi, half * 4 + j),
                                rhs=xq[:, hi, :],
                                start=(hi == 0), stop=(hi == HT - 1),
                            )
                    for j in range(4):
                        nc.vector.tensor_copy(out=qkT[half * 4 + j][:, :],
                                              in_=ps4[j][:, :SQ])

                # transposed scores per 128-key chunk; exp; causal mask
                for kc in range(SC):
                    sp = psum_s.tile([P, 512], F32, tag="sT")
                    for hi in range(HT):
                        nc.tensor.matmul(
                            out=sp[:, :SQ],
                            lhsT=xt[:, hi, kc * P:(kc + 1) * P],
                            rhs=qkT[hi][:, :],
                            start=(hi == 0), stop=(hi == HT - 1),
                        )
                    nc.scalar.activation(
                        pT[kc][:, :], sp[:, :SQ], Exp, scale=SCALE,
                        bias=sb[:, kc:kc + 1] if sb is not None else 0.0)
                    nc.vector.tensor_tensor(
                        out=pT[kc][:, :], in0=pT[kc][:, :],
                        in1=msk[:, kc, :], op=MUL)

                # att1e[q, h] = (P1_un @ x)[q, h]; transposed accum.  The
                # 1/rowsum normalization is deferred to qk2T's copies.
                for ho in range(HT):
                    ps = psum.tile([P, 512], F32, tag="acc")
                    for kc in range(SC):
                        nc.tensor.matmul(
                            out=ps[:, :SQ], lhsT=xs[:, ho, kc, :],
                            rhs=pT[kc][:, :], start=(kc == 0),
                            stop=(kc == SC - 1))
                    nc.vector.tensor_copy(out=att1eT[ho][:, :], in_=ps[:, :SQ])
                make_RR(pT, RR, 1)

            # ---------------- Phase B: cross-attention ----------------
            with tc.tile_pool(name="phB", bufs=1) as pB:

                cb = None
                if has_cb:
                    cb = pB.tile([P, EC], F32, name="cb")
                    nc.sync.dma_start(out=cb[:, :], in_=cbias_d[:, :])

                qk2T = [pB.tile([P, SQ], MMDT, name=f"qk2T{i}")
                        for i in range(HT)]
                p2T = [pB.tile([P, SQ], MMDT, name=f"p2T{i}") for i in range(EC)]
                RR2 = pB.tile([P, SQ], F32, name="RR2")

                # encS [e-part, hc, ec, j] for att2e; encT [h-part, e] for
                # scores.  encT's scope closes first (LIFO) to free SBUF.
                with tc.tile_pool(name="phBeS", bufs=1) as pBs:
                    encS = pBs.tile([P, HT, EC, P], MMDT, name="encS")
                    with tc.tile_wait_until(0.008):
                        nc.sync.dma_start(out=encS[:, :, :, :],
                                          in_=encS_d[:, :, :, :])

                    with tc.tile_pool(name="phBeT", bufs=1) as pBt:
                        encT = pBt.tile([P, HT, ENC], MMDT, name="encT")
                        with tc.tile_wait_until(0.006):
                            nc.scalar.dma_start(out=encT[:, :, :],
                                                in_=encT_d[:, :, :])

                        proj_T(qk2T, w1, lambda hi: att1eT[hi][:, :], rr_t=RR)

                        # transposed cross scores per 128-key (encoder) chunk
                        for ec in range(EC):
                            sp = psum_s.tile([P, 512], F32, tag="sT")
                            for hi in range(HT):
                                nc.tensor.matmul(
                                    out=sp[:, :SQ],
                                    lhsT=encT[:, hi, ec * P:(ec + 1) * P],
                                    rhs=qk2T[hi][:, :],
                                    start=(hi == 0), stop=(hi == HT - 1),
                                )
                            nc.scalar.activation(
                                p2T[ec][:, :], sp[:, :SQ], Exp, scale=SCALE,
                                bias=cb[:, ec:ec + 1] if cb is not None else 0.0)
                        make_RR(p2T, RR2, 2)

                    # att2e[q, h] = (P2_un @ enc) * rr2 for OUR query half,
                    # written straight into att2eT cols [0, SQ) and stored
                    # to DRAM for a pairwise AllGather; phase C's local-first
                    # sweep hides the collective's latency.
                    st_insts = []
                    for ho in range(HT):
                        ps = psum.tile([P, 512], F32, tag="acc")
                        for ec in range(EC):
                            nc.tensor.matmul(
                                out=ps[:, :SQ],
                                lhsT=encS[:, ho, ec, :],
                                rhs=p2T[ec][:, :],
                                start=(ec == 0), stop=(ec == EC - 1),
                            )
                        nc.vector.tensor_tensor(
                            out=att2eT[ho][:, 0:SQ], in0=ps[:, :SQ],
                            in1=RR2[:, :], op=MUL)
                        eng = nc.sync if ho % 2 == 0 else nc.scalar
                        st = eng.dma_start(out=cc_in_d[:, ho, :],
                                           in_=att2eT[ho][:, 0:SQ])
                        st_insts.append(st)
                    cc = nc.gpsimd.collective_compute(
                        "AllGather", mybir.AluOpType.bypass,
                        replica_groups=PAIRS,
                        ins=[cc_in_d.ap()], outs=[cc_out_d.ap()],
                    )
                    for st in st_insts:
                        add_dep_helper(cc.ins, st.ins, True,
                                       "cc reads att2e half stores")
                    # peer section of the gather -> att2eT cols [SQ, 2*SQ);
                    # the section index is the peer's rank, from input data
                    for ho in range(HT):
                        ld = nc.sync.dma_start(
                            out=att2eT[ho][:, SQ:2 * SQ],
                            in_=cc_out_d[rpeer_sv, :, ho, :])
                        add_dep_helper(ld.ins, cc.ins, True,
                                       "att2e load reads gather output")

            att1p.release()
            wbig.release()
            prefetch_w2(N_PRE - len(wpre))

            # ---------------- Phase C: output projection ----------------
            # out = att2e @ W2 (+ bout').  Local-first sweep: the first
            # N_PRE vocab chunks run query chunks 0/1 (locally computed
            # columns) first, hiding the pairwise AllGather latency; their
            # W2 tiles stay resident so the qc2/3 return pass is free.
            GRP = 2
            order = [(vc, qc) for vc in range(N_PRE) for qc in (0, 1)]
            order += [(vc, qc) for vc in range(N_PRE) for qc in (2, 3)]
            order += [(vc, qc) for vc in range(N_PRE, NVC) for qc in range(SC)]
            with tc.tile_pool(name="phC_w", bufs=4) as pW, \
                 tc.tile_pool(name="phC_o", bufs=4) as pO:

                ones_t = None
                if has_bout:
                    ones_t = persist.tile([1, P], MMDT, name="ones")
                    nc.vector.memset(ones_t[:, :], 1.0)

                osb = [None] * SC
                wt = None
                bo = None
                for vc, qc in order:
                    g = vc % GRP
                    if vc < N_PRE:
                        wt = wpre[vc]
                    elif qc == 0:
                        wt = pW.tile([P, HT, NV], MMDT, tag="wt")
                        eng = nc.scalar if vc % 2 == 0 else nc.gpsimd
                        eng.dma_start(out=wt[:, :, :], in_=w2_d[vc, :, :, :])
                    if has_bout and qc == 0:
                        bo = pW.tile([1, NV], MMDT, tag="bo")
                        nc.gpsimd.dma_start(out=bo[:, :],
                                            in_=bout_d[vc * NV:(vc + 1) * NV][None, :])
                    if g == 0:
                        osb[qc] = pO.tile([P, GRP * NV], BF16, tag=f"osb{qc}",
                                          name=f"osb{qc}_{vc}")
                    ps = psum.tile([P, NV], F32, tag="acc")
                    for hi in range(HT):
                        last = (hi == HT - 1) and not has_bout
                        nc.tensor.matmul(
                            out=ps[:, :],
                            lhsT=att2eT[hi][:, qc * P:(qc + 1) * P],
                            rhs=wt[:, hi, :],
                            start=(hi == 0), stop=last,
                        )
                    if has_bout:
                        nc.tensor.matmul(
                            out=ps[:, :], lhsT=ones_t[:, :], rhs=bo[:, :],
                            start=False, stop=True,
                        )
                    nc.vector.tensor_copy(
                        out=osb[qc][:, g * NV:(g + 1) * NV], in_=ps[:, :])
                    if g == GRP - 1:
                        v0 = (vc - g) * NV
                        # final groups alternate stores onto sync (idle by
                        # then) so the last stores drain in parallel
                        eng = (nc.sync if (vc >= 24 and qc % 2 == 1)
                               else nc.scalar)
                        eng.dma_start(
                            out=out_d[qc * P:(qc + 1) * P, v0:v0 + GRP * NV],
                            in_=osb[qc][:, :],
                        )
            wprep.release()
    # phase C's first remote-column matmul waits for all 8 peer att2e
    # sends (each +16 on rdma_rsem); added after TileContext scheduling

    nc.compile()
    return nc


def _retile_w(w):
    """[H, H] -> [128, 2, 8, 512] matching wcol's SBUF layout, contiguous."""
    return np.ascontiguousarray(
        w.reshape(HT, P, 2, 512).transpose(1, 2, 0, 3)).astype(BF16NP)


def _host_prep(inputs):
    """Numpy-side sharding/layout prep. Returns (in_maps, flags)."""
    enc = np.asarray(inputs["encoder_outputs"], dtype=np.float32)
    tok = np.asarray(inputs["target_tokens"]).astype(np.int64)
    tok_emb = np.asarray(inputs["tok_emb"], dtype=np.float32)
    pos_emb = np.asarray(inputs["pos_emb"], dtype=np.float32)
    x0 = tok_emb[tok] + pos_emb[:S][None, :, :]          # [B,S,H]

    W = {k: np.asarray(inputs[k], dtype=np.float32)
         for k in ("Wq_s", "Wk_s", "Wv_s", "Wq_c", "Wk_c", "Wv_c", "Wout")}
    bs = {k: np.asarray(inputs[k], dtype=np.float32)
          for k in ("bq_s", "bk_s", "bv_s", "bq_c", "bk_c", "bv_c", "bout")}

    wqk_c = W["Wq_c"] @ W["Wk_c"].T
    wqks = _retile_w(W["Wq_s"] @ W["Wk_s"].T)
    w1 = _retile_w(W["Wv_s"] @ wqk_c)
    w2 = W["Wv_c"] @ W["Wout"]                           # [H, V] host GEMM

    # exact bias folds: k-side biases are softmax no-ops; v-side biases ride
    # the fused weights into cbias / bout'
    bout_eff = bs["bout"] + bs["bv_c"] @ W["Wout"]
    has_sb = bool(np.any(bs["bq_s"]))
    has_cb = bool(np.any(bs["bq_c"]) or np.any(bs["bv_s"]))
    has_bout = bool(np.any(bout_eff))

    # causal masks in TRANSPOSED coords, per core rank r (q in [r*SQ, ..)):
    # mask[p, kc, j] = 1 iff global query (r*SQ + j) >= global key (kc*128+p)
    jj = np.arange(SQ)[None, None, :]
    pp = np.arange(P)[:, None, None]
    kk = np.arange(SC)[None, :, None]
    masks = [((r * SQ + jj) >= (kk * P + pp)).astype(BF16NP) for r in range(2)]

    in_maps = []
    for c in range(NCORES):
        b, r = c // 2, c % 2
        xb, eb = x0[b], enc[b]
        xh = xb[r * SQ:(r + 1) * SQ]                     # its query half
        # W2 half retiled to [vc, p, hi, j] == the SBUF tile layout
        wh = w2[:, r * VSH:(r + 1) * VSH].reshape(HT, P, NVC, NV)
        w2R = np.ascontiguousarray(wh.transpose(2, 1, 0, 3)).astype(BF16NP)
        m = {
            "xQ": np.ascontiguousarray(
                xh.reshape(SQ, HT, P).transpose(2, 1, 0)).astype(BF16NP),
            "xR": np.ascontiguousarray(
                xb.reshape(S, HT, P).transpose(2, 1, 0)).astype(BF16NP),
            "xS": np.ascontiguousarray(
                xb.reshape(SC, P, HT, P).transpose(1, 2, 0, 3)).astype(BF16NP),
            "encTR": np.ascontiguousarray(
                eb.reshape(ENC, HT, P).transpose(2, 1, 0)).astype(BF16NP),
            "encSR": np.ascontiguousarray(
                eb.reshape(EC, P, HT, P).transpose(1, 2, 0, 3)).astype(BF16NP),
            "maskA": masks[r],
            "peerv": np.array([[(c ^ 1) % 2, 0]], dtype=np.int32),
            "WqkS": wqks, "W1": w1, "W2R": w2R,
        }
        if has_sb:
            sbias = SCALE * ((bs["bq_s"] @ W["Wk_s"].T) @ xb.T)     # [S]
            m["sbias"] = np.ascontiguousarray(
                sbias.reshape(SC, P).T.astype(np.float32))
        if has_cb:
            cbias = SCALE * (((bs["bq_c"] @ W["Wk_c"].T)
                              + bs["bv_s"] @ wqk_c) @ eb.T)         # [ENC]
            m["cbias"] = np.ascontiguousarray(
                cbias.reshape(EC, P).T.astype(np.float32))
        if has_bout:
            m["bout"] = np.ascontiguousarray(
                bout_eff[r * VSH:(r + 1) * VSH]).astype(BF16NP)
        in_maps.append(m)
    return in_maps, (has_sb, has_cb, has_bout)


def assemble_output(results):
    out = np.empty((B, S, V), dtype=np.float32)
    for c in range(NCORES):
        b, vh = c // 2, c % 2
        r = results[c]["out"].astype(np.float32)
        if vh == 1:
            # att2eT cols (hence out rows) are in slot order: own query
            # half first.  Rank 1 owns q [256, 512) -> swap the halves.
            r = np.concatenate([r[SQ:], r[:SQ]], axis=0)
        out[b, :, vh * VSH:(vh + 1) * VSH] = r
    return out


def kernel(**inputs):
    from concourse.bass_utils import run_bass_kernel_spmd
    in_maps, (has_sb, has_cb, has_bout) = _host_prep(inputs)
    nc = build_program(has_sb=has_sb, has_cb=has_cb, has_bout=has_bout)
    res = run_bass_kernel_spmd(nc, in_maps, list(range(NCORES)))
    return assemble_output(res.results)
